# revision 50
# baseline (speedup 1.0000x reference)
"""Capsule-routing kernel for Trainium2 (8 NeuronCores, Bass/Tile).

Problem: nn_ClfCapsule — B=256, INPUT_ATOMS=8, MUL=1024, K=20, O=16, 3 routing
iterations.

u_hat[b,m,k,o] = sum_a W[k,o,a]*xt[b,m,a] (335 MB) is never materialized:
  t[b,k,a]  = sum_m c[m,k] * xt[b,m,a]          (TensorE, contract m=1024)
  s[b,k,o]  = sum_a W[k,o,a] * t[b,k,a]         (DVE mul + TensorE a2-sum)
  v         = squash_over_k(s)
  g[b,k,a]  = sum_o W[k,o,a] * v[b,k,o]         (DVE mul + reduce)
  bU[m,k]   = sum_{b,a} xt[b,m,a] * g[b,k,a]    (TensorE, contract (b,a))
Data-parallel over batch (32/core); bU partials AllReduced after iters 1,2.

v2 perf notes (vs the fp32 v1; cost-model makespan 59.2us -> 35.8us):
- everything bf16: matmuls are 1 PE-cycle/row (vs 4 for fp32), DVE
  tensor_tensor gets the 2x packed mode, DMA payloads halve. PSUM
  accumulation stays fp32. Rel-err gate is 2e-2; measured ~3.9e-3.
- exp on the ACT engine (one 318ns op) instead of a 6-op DVE Horner
  chain; ACT otherwise runs only sqrt, and the framework's Exp<->Sqrt
  table loads (1.28us) hide under PE/DVE work.
- softmax denominator via 8 PSUM-accumulated ones-matmuls (one [1,K] out);
  the 1/denom[k] normalization is DEFERRED to the s16 staging mul
  (k-scales commute with the a-sum), keeping reciprocal+replication off
  the critical chain.  The b_ij mean-over-B (1/256) folds into ACT exp's
  scale argument, so the wg tensor IS ws and no scaling op exists.
- s is computed replicated over the 4 a2 partition groups directly by the
  eb matmul (lhs [128,128] = tile(eye32,(4,4))), so squash runs on 128
  partitions and v needs no replication copies. Iteration 1's
  k-independent t1 is folded into the matmul lhs (eb*t1) instead,
  skipping the sP mul entirely.
- layouts keep the last (innermost) free dim packed wherever a DVE
  tensor_tensor runs, because a stride-0/strided last dim disables the
  2x mode (tensor_reduce never gets 2x, so reduce layouts don't matter).
- HW rules found the hard way: DVE may read at most ONE non-scalar PSUM
  operand per op (even the same tensor twice is rejected); partition
  stride-0 broadcasts are rejected; AluOpType.divide fails ISA checks;
  PSUM accumulation groups must run start->stop contiguously per region
  (interleaving c-major passes across 8 open groups corrupts results);
  accumulating DMA is SWDGE-only and slower end-to-end than HWDGE+DVE add.
"""

import numpy as np

B, A, M = 256, 8, 1024
K, O = 20, 16
NCORES = 8
BLOC = B // NCORES  # 32
MC = M // 128       # 8 m-chunks

_prog_cache = {}
USE_COLLECTIVES = True  # debug switch: False replaces AllReduce with local copy


def _build_program(n_reps=1):
    """n_reps > 1 replicates the computation inside one NEFF for
    wall-clock benchmarking (chained bass_exec calls get CSE'd by XLA)."""
    import concourse.bacc as bacc
    import concourse.mybir as mybir
    import concourse.tile as tile

    f32 = mybir.dt.float32
    bf = mybir.dt.bfloat16
    nc = bacc.Bacc("TRN2", target_bir_lowering=False, debug=False,
                   num_devices=NCORES)

    # Host-prepped per-core DRAM inputs (see _host_prep / _host_w):
    #   xm[p, mc, q]       = xt[b, mc*128+p, a]   with q = a*32+b (m on parts)
    #   xf[(a2,b), c, m]   = xt[b, m, c*4+a2]
    #   ws[(a2,b), c, o, k] = W[k, o, c*4+a2]
    #   eb[(a2,b), (a2',b')] = (b == b')  (s replicated over a2 groups via PE)
    xm_d = nc.dram_tensor("xm", [128, MC, 256], bf, kind="ExternalInput")
    xf_d = nc.dram_tensor("xf", [128, 2, M], bf, kind="ExternalInput")
    ws_d = nc.dram_tensor("ws", [128, 2, O, K], bf, kind="ExternalInput")
    eb_d = nc.dram_tensor("eb", [128, 128], bf, kind="ExternalInput")
    out_d = nc.dram_tensor("out", [BLOC, K, O], f32, kind="ExternalOutput")

    EXP = mybir.ActivationFunctionType.Exp
    ADD = mybir.AluOpType.add
    MULT = mybir.AluOpType.mult
    DIV = mybir.AluOpType.divide
    AXX = mybir.AxisListType.X
    RG = [list(range(NCORES))]

    with tile.TileContext(nc) as tc:
        with (
            nc.allow_low_precision(reason="bf16 glue; rel-err gate is 2e-2"),
            tc.tile_pool(name="const", bufs=1) as cpool,
            tc.tile_pool(name="work", bufs=2) as wpool,
            tc.tile_pool(name="ps_tt", bufs=1, space="PSUM") as ps_tt,
            tc.tile_pool(name="ps_s", bufs=1, space="PSUM") as ps_s,
            tc.tile_pool(name="ps_bu", bufs=1, space="PSUM") as ps_bu,
            tc.tile_pool(name="dram", bufs=2, space="DRAM") as dpool,
        ):
            ones128 = cpool.tile([128, 1], bf)
            onesM = cpool.tile([128, 1], bf)  # 1/M: pre-scales the rowsum
            ones1 = cpool.tile([1, 128], f32)
            eb_sb = cpool.tile([128, 128], bf)
            nc.vector.memset(ones128[:], 1.0)
            nc.vector.memset(onesM[:], 1.0 / M)
            nc.vector.memset(ones1[:], 1.0)
            nc.gpsimd.dma_start(eb_sb[:], eb_d[:])

            for _rep in range(n_reps):
                xm_sb = wpool.tile([128, MC, 256], bf, name="xm_sb")
                xf_sb = wpool.tile([128, 2, M], bf, name="xf_sb")
                ws_sb = wpool.tile([128, 2, O, K], bf, name="ws_sb")
                # xm halves on two parallel queues (SP + ACT HWDGEs); the
                # small ws before xf on the serial gpsimd/SWDGE queue so
                # iter-1's sP mul isn't gated on the big xf transfer.
                nc.sync.dma_start(xm_sb[:, 0:MC // 2], xm_d[:, 0:MC // 2])
                nc.scalar.dma_start(xm_sb[:, MC // 2:], xm_d[:, MC // 2:])
                nc.gpsimd.dma_start(ws_sb[:], ws_d[:])
                nc.gpsimd.dma_start(xf_sb[:], xf_d[:])

                ar_sbs = []
                for it in range(3):
                    first = (it == 0)
                    # ---- softmax over m + t[b,k,a] (iter 1: b=0, uniform) --
                    if first:
                        # t1 = rowsum(x)/M via 1/M-ones matmuls; k-indep.
                        tt = ps_tt.tile([128, 2, 1], f32, name="tt0", tag="tt")
                        for c in range(2):
                            for mc in range(MC):
                                nc.tensor.matmul(
                                    tt[:, c, :],
                                    xm_sb[:, mc, c * 128:(c + 1) * 128],
                                    onesM[:],
                                    start=(mc == 0), stop=(mc == MC - 1))
                        # t1 is k-independent: fold it into the eb lhs
                        # (lhsA_c = eb * t1[:,c]) so the s matmuls take ws
                        # directly as rhs — skips the [128,2,O,K] sP mul.
                        lhsA = wpool.tile([128, 2, 128], bf, name="lhsA0")
                        nc.vector.tensor_tensor(
                            lhsA[:], eb_sb[:].unsqueeze(1)
                            .broadcast_to([128, 2, 128]),
                            tt[:, :, 0].unsqueeze(2)
                            .broadcast_to([128, 2, 128]),
                            op=MULT)
                    else:
                        # it==1 reads ar0 = bU1; it==2 reads ar1 which IS
                        # b2 already (each core pre-added ar0/8 into its
                        # AR2 input during staging, so the AllReduce sums
                        # 8*(ar0/8) = ar0 back in).
                        e_src = ar_sbs[it - 1]
                        # exp(bU/B): the mean-over-B folds into ACT's scale.
                        e_sb = wpool.tile([128, MC, K], bf, name=f"e{it}")
                        nc.scalar.activation(e_sb[:], e_src[:], EXP,
                                             scale=1.0 / B)
                        # t (unnormalized): 16 matmuls, contract m.
                        # Emitted before the denominator matmuls: the rcp
                        # path has slack until the s16 mul, t4c does not.
                        tt = ps_tt.tile([128, 2, K], f32, name=f"tt{it}",
                                        tag="tt")
                        for c in range(2):
                            for mc in range(MC):
                                nc.tensor.matmul(
                                    tt[:, c, :],
                                    xm_sb[:, mc, c * 128:(c + 1) * 128],
                                    e_sb[:, mc, :],
                                    start=(mc == 0), stop=(mc == MC - 1))
                        # denom[k] = sum_m e via 8 PSUM-accumulated matmuls
                        dn = ps_s.tile([1, K], f32, name=f"dn{it}", tag="dn")
                        for mc in range(MC):
                            nc.tensor.matmul(dn[:], ones128[:], e_sb[:, mc, :],
                                             start=(mc == 0), stop=(mc == MC - 1))
                        rcp = wpool.tile([1, K], f32, name=f"rcp{it}")
                        nc.vector.reciprocal(rcp[:], dn[:])
                        # replicate 1/denom to all partitions via PE
                        rb = ps_s.tile([128, K], f32, name=f"rb{it}", tag="rb")
                        nc.tensor.matmul(rb[:], ones1[:], rcp[:],
                                         start=True, stop=True)
                        rb16 = wpool.tile([128, K], bf, name=f"rb16_{it}")
                        nc.vector.tensor_copy(rb16[:], rb[:])
                        # normalization by 1/denom[k] is deferred to the s16
                        # staging mul (k-scales commute with the a-sum), so
                        # the rcp/rb path is off the critical chain here.
                        # Split by c: c0's copy/mul overlap c1's t matmuls.
                        t4c = wpool.tile([128, 2, K], bf, name=f"t4c{it}")
                        for c in range(2):
                            nc.vector.tensor_copy(t4c[:, c], tt[:, c])

                    # ---- s[b,k,o] = sum_a W*t: DVE mul + eb matmul --------
                    s128 = ps_s.tile([128, O, K], f32, name=f"s{it}", tag="s")
                    if first:
                        for c in range(2):
                            nc.tensor.matmul(s128[:], lhsA[:, c], ws_sb[:, c],
                                             start=(c == 0), stop=(c == 1))
                    else:
                        sP = wpool.tile([128, 2, O, K], bf, name=f"sP{it}")
                        for c in range(2):
                            nc.vector.tensor_tensor(
                                sP[:, c],
                                t4c[:, c].unsqueeze(1)
                                .broadcast_to([128, O, K]),
                                ws_sb[:, c], op=MULT)
                        for c in range(2):
                            nc.tensor.matmul(s128[:], eb_sb[:], sP[:, c],
                                             start=(c == 0), stop=(c == 1))

                    # ---- squash over k (on all 128 partitions) ------------
                    # s16: SBUF staging of s (DVE cannot read PSUM twice for
                    # s*s, and keeping ACT exp-mostly lets table loads hide).
                    # For softmax iters the deferred 1/denom[k] rides along.
                    s16 = wpool.tile([128, O, K], bf, name=f"s16_{it}")
                    sq = wpool.tile([128, O, K], bf, name=f"sq{it}")
                    if first:
                        nc.vector.tensor_copy(s16[:], s128[:])
                    else:
                        nc.vector.tensor_tensor(
                            s16[:], s128[:],
                            rb16[:].unsqueeze(1).broadcast_to([128, O, K]),
                            op=MULT)
                    nc.vector.tensor_tensor(sq[:], s16[:], s16[:], op=MULT)
                    ms = wpool.tile([128, O], f32, name=f"ms{it}")
                    nc.vector.tensor_reduce(ms[:], sq[:], axis=AXX, op=ADD)
                    mag = wpool.tile([128, O], f32, name=f"mag{it}")
                    nc.scalar.sqrt(mag[:], ms[:])
                    den = wpool.tile([128, O], f32, name=f"den{it}")
                    nc.vector.tensor_scalar_add(den[:], ms[:], 1.0)
                    rd = wpool.tile([128, O], f32, name=f"rd{it}")
                    nc.vector.reciprocal(rd[:], den[:])
                    fq = wpool.tile([128, O], f32, name=f"fq{it}")
                    nc.vector.tensor_tensor(fq[:], mag[:], rd[:], op=MULT)

                    if it == 2:
                        vout = wpool.tile([BLOC, K, O], f32, name="vout")
                        nc.vector.tensor_tensor(
                            vout[:].transpose([0, 2, 1]), s16[0:BLOC],
                            fq[0:BLOC].unsqueeze(2).broadcast_to([BLOC, O, K]),
                            op=MULT)
                        nc.sync.dma_start(out_d[:], vout[:])
                        continue

                    v128 = wpool.tile([128, O, K], bf, name=f"v{it}")
                    nc.vector.tensor_tensor(
                        v128[:], s16[:],
                        fq[:].unsqueeze(2).broadcast_to([128, O, K]), op=MULT)

                    # ---- g[b,k,a]/1: gP mul + o-reduce (wg == ws) ---------
                    gP = wpool.tile([128, 2, O, K], bf, name=f"gP{it}")
                    Gp = wpool.tile([128, 2, K], bf, name=f"Gp{it}")
                    for c in range(2):
                        nc.vector.tensor_tensor(
                            gP[:, c], v128[:], ws_sb[:, c], op=MULT)
                        nc.vector.tensor_reduce(
                            Gp[:, c], gP[:, c].transpose([0, 2, 1]),
                            axis=AXX, op=ADD)

                    # ---- matmul2 (c-major): bU[m,k] = sum_q Xf[q,m] Gp[q,k]
                    # two PSUM tiles so the first-half bf16 staging copy
                    # overlaps the second half's matmuls (one tile would
                    # serialize them through tile-level dependencies)
                    H = MC // 2
                    bu_a = ps_bu.tile([128, H, K], f32, name=f"bua{it}",
                                      tag="bua")
                    bu_b = ps_bu.tile([128, H, K], f32, name=f"bub{it}",
                                      tag="bub")
                    bu16 = wpool.tile([128, MC, K], bf, name=f"bu16_{it}")
                    for mt in range(MC):
                        half = bu_a if mt < H else bu_b
                        for c in range(2):
                            nc.tensor.matmul(
                                half[:, mt % H, :],
                                xf_sb[:, c, mt * 128:(mt + 1) * 128],
                                Gp[:, c, :],
                                start=(c == 0), stop=(c == 1))
                        if mt == H - 1:
                            if it == 0:
                                # first half staged by ACT (Copy is in every
                                # table set, never forces a load); second by
                                # DVE — the two run in parallel
                                nc.scalar.copy(bu16[:, 0:H], bu_a[:])
                            else:
                                # fold b2 = bU1 + bU2 into the staging: each
                                # core adds ar0/NCORES locally, the AllReduce
                                # restores ar0 once — removes the post-AR add
                                # from iter 3's critical path
                                nc.vector.scalar_tensor_tensor(
                                    bu16[:, 0:H], ar_sbs[0][:, 0:H],
                                    1.0 / NCORES, bu_a[:],
                                    op0=MULT, op1=ADD)
                    if it == 0:
                        nc.vector.tensor_copy(bu16[:, H:], bu_b[:])
                    else:
                        nc.vector.scalar_tensor_tensor(
                            bu16[:, H:], ar_sbs[0][:, H:], 1.0 / NCORES,
                            bu_b[:], op0=MULT, op1=ADD)

                    # ---- AllReduce the (unscaled) b_ij update, in bf16 ----
                    cc_in = dpool.tile([128, MC, K], bf, name=f"cci{it}")
                    cc_out = dpool.tile([128, MC, K], bf, name=f"cco{it}",
                                        addr_space="Shared")
                    nc.sync.dma_start(cc_in[:], bu16[:])
                    if USE_COLLECTIVES:
                        nc.gpsimd.collective_compute(
                            "AllReduce", ADD, replica_groups=RG,
                            ins=[cc_in[:].opt()], outs=[cc_out[:].opt()])
                        ar_src = cc_out
                    else:
                        # collective-free build for TimelineSim: the whole
                        # AllReduce op (incl. its internal DMA phases) is
                        # charged separately by test.py's estimate, so no
                        # stand-in copy here — hop3 reads cc_in back.
                        ar_src = cc_in
                    ar_sb = wpool.tile([128, MC, K], bf, name=f"ar{it}")
                    nc.sync.dma_start(ar_sb[:], ar_src[:])
                    ar_sbs.append(ar_sb)

    nc.compile()
    return nc


def _host_prep(x):
    """Build the 8 per-core input maps from the full x [B, A, M]."""
    import ml_dtypes
    bf = ml_dtypes.bfloat16
    x = np.ascontiguousarray(x, dtype=np.float32)
    xt = x.reshape(B, M, A)  # faithful to reference's reshape (NOT a transpose)
    in_maps = []
    for i in range(NCORES):
        xi = xt[i * BLOC:(i + 1) * BLOC]              # [32, 1024, 8]
        # xm[p, mc, a*32+b]
        xm = xi.transpose(1, 2, 0).reshape(MC, 128, A, BLOC)
        xm = np.ascontiguousarray(xm.transpose(1, 0, 2, 3)).reshape(128, MC, 256)
        # xf[a2*32+b, c, m] with a = c*4+a2
        xf = xi.transpose(2, 0, 1).reshape(2, 4, BLOC, M)
        xf = np.ascontiguousarray(xf.transpose(1, 2, 0, 3)).reshape(128, 2, M)
        in_maps.append({"xm": xm.astype(bf), "xf": xf.astype(bf)})
    return in_maps


def _host_w(W):
    """ws[(a2,b), c, o, k] = W[k, o, c*4+a2]; eb = tile(eye(32), (4,4))."""
    import ml_dtypes
    bf = ml_dtypes.bfloat16
    W = np.ascontiguousarray(W, dtype=np.float32)
    wss = W.reshape(K, O, 2, 4).transpose(3, 2, 1, 0)    # [a2, c, o, k]
    ws = np.ascontiguousarray(
        np.broadcast_to(wss[:, None], (4, BLOC, 2, O, K))).reshape(
            128, 2, O, K)
    eb = np.tile(np.eye(BLOC, dtype=np.float32), (4, 4))
    return {"ws": ws.astype(bf), "eb": eb.astype(bf)}


def _run(x, W, trace=False):
    from concourse import bass_utils

    if "nc" not in _prog_cache:
        _prog_cache["nc"] = _build_program()
    nc = _prog_cache["nc"]

    consts = _host_w(W)
    in_maps = _host_prep(x)
    for m in in_maps:
        m.update(consts)

    res = bass_utils.run_bass_kernel_spmd(
        nc, in_maps, core_ids=list(range(NCORES)), trace=trace)
    out = np.concatenate([r["out"] for r in res.results], axis=0)
    return out.reshape(B, K, O, 1).astype(np.float32), res


def kernel(x, W):
    out, _ = _run(x, W)
    return out
